# revision 46
# baseline (speedup 1.0000x reference)
"""Trainium2 Bass kernel for nn_AttentionBlock (B=4, H=W=64, C=256, D=32).

Sharding: 8 shards = 4 samples x 2 query-halves. Each core gets the full
sample's rows (reordered so its 2048 query rows come first), computes K for
all 4096 keys, and attention for its 2048 queries. No collectives.

v6 algorithm (projection folding + fp8 end-to-end):
  device: dev[q,c] = (1/d[q]) * (G^T @ W2)[q,c];  host: out = x + dev
  W2 = 32 * wv @ wo   (host precompute)
  G[c,q] = sum_k x8[k,c] * E8[k,q]   (fp8 DoubleRow matmuls, contraction 256)
  E8 = fp8e5m2(exp(S/256 - 2)),  S = K Q^T scores at x256 scale (wq,wk
       stored e4m3 with x16 scale each; compensated in the exp scale)
  d[q] = sum_k E8[k,q]       (col-packed ones matmuls + transpose matmul)
exp computed two ways in parallel: ACT true exp -> e5m2, and DVE integer
bit-trick (Schraudolph in e5m2 space: bits = (5.7708/256)*S + 48.76).
S matmuls (contraction D=32) use 4-way PE row-tiling via tile_position.
Q/K projections use DoubleRow. HAM warmup matmuls bridge the ~9us DMA
startup dead zone; phases A/B interleave; epilogue spread across steps.

Self-contained: hardcodes shapes, imports only /opt/trn_rl_repo concourse.
"""

import sys

if "/opt/trn_rl_repo" not in sys.path:
    sys.path.insert(0, "/opt/trn_rl_repo")

import numpy as np
import ml_dtypes

BF16 = ml_dtypes.bfloat16
E4M3 = ml_dtypes.float8_e4m3
E5M2 = ml_dtypes.float8_e5m2

# Problem constants
B, HH, WW, C = 4, 64, 64, 256
D = 32
N = HH * WW           # 4096 keys per sample
NQ = N // 2           # 2048 queries per core
NCORES = 8
KC = N // 128         # 32 key chunks
NG = NQ // 512        # 4 query groups of 512 per core
NSTEP = 8             # 4-chunk steps per query group (32 chunks / 4)
PIPE = 3              # consume s - PIPE

C0 = 4.77             # exp shift: weights = exp(S - C0), cancels in softmax
SS = 256.0            # scores arrive at x256 scale (wq,wk each x16 in fp8)
EXP_A = 5.770780 / SS  # 4*log2(e) / 256
EXP_B = 60.0 + 0.3 - 5.770780 * C0  # e5m2 bias 60, +0.3 truncation recenter
NWARM = 60            # HAM warmup matmuls (bridge ~9us DMA startup)

_compiled_cache = {}


def _build():
    from contextlib import ExitStack
    from concourse import bacc, tile, mybir, masks

    f32 = mybir.dt.float32
    bf = mybir.dt.bfloat16
    fp8e4 = mybir.dt.float8e4
    fp8e5 = mybir.dt.float8e5
    u8 = mybir.dt.uint8

    nc = bacc.Bacc("TRN2", target_bir_lowering=False, debug=False, num_devices=NCORES)

    x8_d = nc.dram_tensor("x8p", [128, KC, 256], fp8e4, kind="ExternalInput")
    xT8_d = nc.dram_tensor("xT8p", [128, 2, N], fp8e4, kind="ExternalInput")
    wq_d = nc.dram_tensor("wq8p", [128, 256], fp8e4, kind="ExternalInput")
    wk_d = nc.dram_tensor("wk8p", [128, 256], fp8e4, kind="ExternalInput")
    bq_d = nc.dram_tensor("bq_col", [128, 1], f32, kind="ExternalInput")
    bk_d = nc.dram_tensor("bk_col", [128, 1], f32, kind="ExternalInput")
    w2_d = nc.dram_tensor("w28p", [128, 512], fp8e4, kind="ExternalInput")
    out_d = nc.dram_tensor("out", [NQ, C], bf, kind="ExternalOutput")

    Exp = mybir.ActivationFunctionType.Exp
    Add = mybir.AluOpType.add
    Mult = mybir.AluOpType.mult
    DR = mybir.MatmulPerfMode.DoubleRow

    with tile.TileContext(nc) as tc:
        with ExitStack() as ctx:
            const = ctx.enter_context(tc.tile_pool(name="const", bufs=1))
            big = ctx.enter_context(tc.tile_pool(name="big", bufs=1))
            expp = ctx.enter_context(tc.tile_pool(name="expp", bufs=12))
            small = ctx.enter_context(tc.tile_pool(name="small", bufs=2))
            ps_s = ctx.enter_context(tc.tile_pool(name="ps_s", bufs=2, space="PSUM"))
            ps_g = ctx.enter_context(tc.tile_pool(name="ps_g", bufs=1, space="PSUM"))
            ps_d = ctx.enter_context(tc.tile_pool(name="ps_d", bufs=1, space="PSUM"))
            ps_e = ctx.enter_context(tc.tile_pool(name="ps_e", bufs=1, space="PSUM"))

            # ---- consts: warmup weights first (plain memset, fast), then identity ----
            warm8 = const.tile([128, 128], fp8e4, tag="warm8")
            nc.gpsimd.memset(warm8[:], 1.0)
            ones8 = const.tile([128, 32], fp8e5, tag="ones8")
            nc.gpsimd.memset(ones8[:], 1.0)
            ones1 = const.tile([128, 1], bf, tag="ones1")
            nc.gpsimd.memset(ones1[:], 1.0)
            negc0 = const.tile([128, 1], f32, tag="negc0")
            nc.gpsimd.memset(negc0[:], -C0)

            # ---- input DMAs: x8 split across both HWDGE rings, fat descriptors ----
            x8sb = big.tile([128, KC, 256], fp8e4, tag="x8sb")
            wqsb = const.tile([128, 2, 128], fp8e4, tag="wqsb")
            wksb = const.tile([128, 2, 128], fp8e4, tag="wksb")
            w2sb = const.tile([128, 2, 256], fp8e4, tag="w2sb")
            bqc = const.tile([128, 1], f32, tag="bqc")
            bkc = const.tile([128, 1], f32, tag="bkc")

            xT8 = big.tile([128, 2, N], fp8e4, tag="xT8")
            nc.sync.dma_start(out=wqsb[:], in_=wq_d[:].rearrange("p (j m) -> p j m", j=2))
            nc.sync.dma_start(out=wksb[:], in_=wk_d[:].rearrange("p (j m) -> p j m", j=2))
            nc.sync.dma_start(out=bqc[:], in_=bq_d[:])
            nc.sync.dma_start(out=bkc[:], in_=bk_d[:])
            nc.sync.dma_start(out=xT8[:, :, 0:2048], in_=xT8_d[:, :, 0:2048])
            nc.sync.dma_start(out=xT8[:, :, 2048:4096], in_=xT8_d[:, :, 2048:4096])
            nc.sync.dma_start(out=x8sb[:, 0:16, :], in_=x8_d[:, 0:16, :])
            nc.sync.dma_start(out=x8sb[:, 16:32, :], in_=x8_d[:, 16:32, :])
            nc.sync.dma_start(out=w2sb[:], in_=w2_d[:].rearrange("p (j m) -> p j m", j=2))

            # ---- HAM warmup: dense dummy matmuls while DMAs land ----
            wmt = ps_s.tile([128, 2, 512], f32, tag="s", name="warm")
            for i in range(NWARM):
                nc.tensor.matmul(wmt[:, 0, 0:128], warm8[:], warm8[:], start=True, stop=True)
            # pre-load ACT exp table
            dumm = const.tile([128, 1], f32, tag="dumm")
            nc.scalar.activation(dumm[:], negc0[:], Exp, bias=negc0[:])

            # ---- phase B: qT/kT replicated x4 along partitions, DoubleRow.
            # Casts split DVE/scalar; filler matmuls keep the PE dense.
            qT = big.tile([128, NQ], bf, tag="qT")
            kT = big.tile([128, N], bf, tag="kT")

            pcount = [0]

            def project(nm, dst, w, bias, s, home):
                # projections alternate between the S pool and the (idle until
                # phase C) denom bank: two independent psum chains pipeline
                if home == "d":
                    pjt = ps_d.tile([128, 512], f32, tag="d", name=f"pj{nm}{s}")
                    pout = pjt[:]
                    filt = None
                else:
                    pjt = ps_s.tile([128, 2, 512], f32, tag="s", name=f"pj{nm}{s}")
                    pout = pjt[:, 0, :]
                    filt = pjt[:, 1, 0:128]
                nc.tensor.matmul(pout, w[:], xT8[:, :, 512 * s : 512 * s + 512],
                                 start=True, stop=True, perf_mode=DR)
                dv = dst[:, 512 * s : 512 * s + 512]
                if pcount[0] % 2 == 0:
                    nc.vector.tensor_scalar(dv, pout, bias, None, Add)
                else:
                    nc.scalar.activation(dv, pout,
                                         mybir.ActivationFunctionType.Identity, bias=bias)
                pcount[0] += 1
                # fillers into the unused psum half: keep the PE dense for HAM
                if filt is not None:
                    for i in range(2):
                        nc.tensor.matmul(filt, warm8[:], warm8[:], start=True, stop=True)

            for s in range(8):
                project("k", kT, wksb, bkc, s, "d" if s % 2 == 1 else "s")
            project("q", qT, wqsb, bqc, 0, "s")

            # ---- phase C: flat pipeline over 32 steps of 4 key chunks ----
            sts = {}
            ets = {}
            gtile = {}
            dtile = {}
            ottile = {}
            epi_state = {}  # group -> (gsb, rec, ott)

            def produce(s):
                g, t = divmod(s, NSTEP)
                if t == 0:
                    gtile[g] = ps_g.tile([128, 2, 512], f32, tag="g", name=f"g{g}")
                    ottile[g] = small.tile([128, 4, 256], bf, tag="ot", name=f"ot{g}")
                if s < 3:
                    project("q", qT, wqsb, bqc, s + 1, "d")
                if s < PIPE + 2:
                    fil = ps_e.tile([128, 256], f32, tag="er", name=f"fil{s}")
                    for i in range(8):
                        nc.tensor.matmul(fil[:, 0:128], warm8[:], warm8[:], start=True, stop=True)
                sA = ps_s.tile([128, 2, 512], f32, tag="s", name=f"sA{s}")
                sB = ps_s.tile([128, 2, 512], f32, tag="s", name=f"sB{s}")
                for i in range(4):
                    m = 4 * t + i
                    nc.tensor.matmul(
                        (sA if i < 2 else sB)[:, i % 2, :],
                        kT[32 * i : 32 * i + 32, 128 * m : 128 * m + 128],
                        qT[32 * i : 32 * i + 32, 512 * g : 512 * g + 512],
                        start=True,
                        stop=True,
                        tile_position=(32 * i, 0),
                    )
                etA = expp.tile([128, 2, 512], fp8e5, tag="e", name=f"eA{s}")
                etB = expp.tile([128, 2, 512], fp8e5, tag="e", name=f"eB{s}")
                nc.scalar.activation(etA[:], sA[:], Exp, bias=negc0[:], scale=1.0 / SS)
                boundary = (s - PIPE) % NSTEP == NSTEP - 1 and s >= PIPE
                if boundary:
                    # epilogue runs on DVE this step: shift exp load to ACT
                    nc.scalar.activation(etB[:, 0, :], sB[:, 0, :], Exp, bias=negc0[:], scale=1.0 / SS)
                    nc.scalar.activation(etB[:, 1, :], sB[:, 1, :], Exp, bias=negc0[:], scale=1.0 / SS)
                else:
                    nc.vector.tensor_scalar(
                        etB[:].bitcast(u8), sB[:], EXP_A, EXP_B, Mult, Add
                    )
                sts[s] = (sA, sB)
                ets[s] = (etA, etB)

            def consume(s):
                g, t = divmod(s, NSTEP)
                etA, etB = ets.pop(s)
                sts.pop(s)
                gp = gtile[g]
                if t == 0:
                    dtile[g] = ps_d.tile([128, 512], f32, tag="d", name=f"d{g}")
                dp = dtile[g]
                for pa, et in ((0, etA), (1, etB)):
                    pp = 2 * t + pa
                    for h in range(2):
                        nc.tensor.matmul(
                            gp[:, h, :],
                            x8sb[:, 4 * t + 2 * pa : 4 * t + 2 * pa + 2, 128 * h : 128 * h + 128],
                            et[:],
                            start=(pp == 0),
                            stop=(pp == 2 * NSTEP - 1),
                            perf_mode=DR,
                        )
                for j in range(4):
                    et = (etA if j < 2 else etB)
                    nc.tensor.matmul(
                        dp[32 * j : 32 * j + 32, :],
                        ones8[:],
                        et[:, j % 2, :],
                        start=(t == 0),
                        stop=(t == NSTEP - 1),
                        tile_position=(0, 32 * j),
                    )

            def epi_head(g):
                """Group boundary: free G/denom psum via casts, compute rec."""
                gp = gtile.pop(g)
                dp = dtile.pop(g)
                dsb = small.tile([128, 512], bf, tag="dsb")
                nc.vector.tensor_copy(dsb[:], dp[:])
                gsb = small.tile([128, 2, 512], fp8e4, tag="gsb")
                nc.scalar.copy(gsb[:, 0, :], gp[:, 0, :])
                nc.vector.tensor_copy(gsb[:, 1, :], gp[:, 1, :])
                rp = ps_d.tile([128, 512], f32, tag="d", name=f"rp{g}")
                for b in range(4):
                    nc.tensor.matmul(
                        rp[:, b : b + 1], dsb[:, 128 * b : 128 * b + 128],
                        ones1[:], start=True, stop=True,
                    )
                rec = small.tile([128, 4], f32, tag="recs")
                nc.vector.reciprocal(rec[:], rp[:, 0:4])
                epi_state[g] = (gsb, rec, ottile.pop(g), None)

            def epi_block(g, b):
                gsb, rec, ott, eplast = epi_state[g]
                if g == NG - 1:
                    # last group: no later compute hides the chain; rotate
                    # across the er bank and the two freed G banks so the
                    # per-block matmuls pipeline without false deps
                    if eplast is None:
                        eplast = ps_g.tile([128, 2, 512], f32, tag="g", name="glast")
                        epi_state[g] = (gsb, rec, ott, eplast)
                    if b % 3 == 2:
                        ept = ps_e.tile([128, 256], f32, tag="er", name=f"erL{b}")
                        ep = ept[:]
                    else:
                        ep = eplast[:, b % 3, 0:256]
                else:
                    ept = ps_e.tile([128, 256], f32, tag="er", name=f"er{g}_{b}")
                    ep = ept[:]
                nc.tensor.matmul(ep, gsb[:, :, 128 * b : 128 * b + 128], w2sb[:],
                                 start=True, stop=True, perf_mode=DR)
                if b % 2 == 0:
                    nc.vector.tensor_scalar(ott[:, b, :], ep, rec[:, b : b + 1], None, Mult)
                else:
                    nc.scalar.mul(ott[:, b, :], ep, rec[:, b : b + 1])
                out_r = out_d[:].rearrange("(t p) c -> p t c", p=128)
                if g == NG - 1:
                    nc.sync.dma_start(
                        out=out_r[:, 4 * g + b : 4 * g + b + 1, :], in_=ott[:, b : b + 1, :]
                    )
                elif b == 3:
                    nc.sync.dma_start(out=out_r[:, 4 * g : 4 * g + 4, :], in_=ott[:])
                if b == 3:
                    del epi_state[g]

            for s in range(NG * NSTEP + PIPE + 4):
                if s >= PIPE and s - PIPE < NG * NSTEP:
                    sc = s - PIPE
                    consume(sc)
                    if sc % NSTEP == NSTEP - 1:
                        epi_head(sc // NSTEP)
                # spread epilogue blocks: block b of group g at flat-step
                # (boundary of g) + 1 + b; last group bursts immediately
                for g in range(NG - 1):
                    bnd = 8 * g + 7 + PIPE  # flat-step of epi_head(g)
                    b = s - bnd - 1
                    if 0 <= b < 4:
                        epi_block(g, b)
                if s == 8 * (NG - 1) + 7 + PIPE:
                    for b in range(4):
                        epi_block(NG - 1, b)
                if s < NG * NSTEP:
                    produce(s)

    nc.compile()
    return nc


def _get_compiled():
    if "v6" not in _compiled_cache:
        _compiled_cache["v6"] = _build()
    return _compiled_cache["v6"]


def _prep(x, wq, bq, wk, bk, wv, bv, wo, bo):
    xf = np.ascontiguousarray(np.asarray(x, dtype=np.float32)).reshape(B, N, C)
    wq = np.asarray(wq, np.float32)
    bq = np.asarray(bq, np.float32)
    wk = np.asarray(wk, np.float32)
    bk = np.asarray(bk, np.float32)
    wv = np.asarray(wv, np.float32)
    bv = np.asarray(bv, np.float32)
    wo = np.asarray(wo, np.float32)
    bo = np.asarray(bo, np.float32)

    scale = np.float32(1.0 / np.sqrt(np.float32(D)))
    # wq,wk stored e4m3 at x16 scale each (S comes out x256; exp rescales),
    # replicated x4 along d, then packed [128, 2, 128] for DoubleRow.
    wq_rep = np.tile(wq * (scale * 16.0), (1, 4)).astype(E4M3)  # [256, 128]
    wk_rep = np.tile(wk * 16.0, (1, 4)).astype(E4M3)
    wq8p = np.ascontiguousarray(wq_rep.reshape(2, 128, 128).transpose(1, 0, 2).reshape(128, 256))
    wk8p = np.ascontiguousarray(wk_rep.reshape(2, 128, 128).transpose(1, 0, 2).reshape(128, 256))
    bq_col = np.ascontiguousarray(np.tile(bq * (scale * 16.0), 4)[:, None]).astype(np.float32)
    bk_col = np.ascontiguousarray(np.tile(bk * 16.0, 4)[:, None]).astype(np.float32)
    w2f = (32.0 * (wv @ wo)).astype(E4M3)  # x32 folds denom replication
    w2 = np.ascontiguousarray(w2f.reshape(2, 128, 256).transpose(1, 0, 2).reshape(128, 512))
    r0 = (bv @ wo + bo).astype(np.float32)  # residual-folded constant bias row

    in_maps = []
    for core in range(NCORES):
        b, h = divmod(core, 2)
        if h == 0:
            xo = xf[b]
        else:
            xo = np.concatenate([xf[b, NQ:], xf[b, :NQ]], 0)
        # partition-major layouts with 8KB contiguous per partition
        xo8 = xo.astype(E4M3)
        x8p = np.ascontiguousarray(xo8.reshape(KC, 128, 256).transpose(1, 0, 2))
        xT8p = np.ascontiguousarray(xo8.T.reshape(2, 128, N).transpose(1, 0, 2))
        in_maps.append(
            {
                "x8p": x8p,
                "xT8p": xT8p,
                "wq8p": wq8p,
                "wk8p": wk8p,
                "bq_col": bq_col,
                "bk_col": bk_col,
                "w28p": w2,
            }
        )
    return in_maps, r0


def _gather(results, xf, r0):
    out = np.empty((B, N, C), np.float32)
    for core in range(NCORES):
        b, h = divmod(core, 2)
        out[b, NQ * h : NQ * (h + 1)] = results[core]["out"].astype(np.float32)
    out += xf + r0[None, None, :]  # residual + folded output bias (host side)
    return out.reshape(B, HH, WW, C)


def kernel(x, wq, bq, wk, bk, wv, bv, wo, bo):
    from concourse.bass_utils import run_bass_kernel_spmd

    in_maps, r0 = _prep(x, wq, bq, wk, bk, wv, bv, wo, bo)
    nc = _get_compiled()
    res = run_bass_kernel_spmd(nc, in_maps, core_ids=list(range(NCORES)))
    xf = np.asarray(x, dtype=np.float32).reshape(B, N, C)
    return _gather(res.results, xf, r0)


def _ensure_ntff_hook():
    """The agent image's antenv stub lacks axon_hooks; synthesize it so
    run_bass_kernel_spmd(trace=True) can NTFF-profile via libaxon_pjrt."""
    import types

    try:
        from antenv.axon_hooks import get_axon_ntff_profile_hook  # noqa: F401
        return
    except ImportError:
        pass
    import antenv
    from trn_agent_boot.trn_boot import _ntff_profile_via_ctypes

    mod = types.ModuleType("antenv.axon_hooks")
    state = {"h": _ntff_profile_via_ctypes("/opt/axon/libaxon_pjrt.so")}
    mod.get_axon_ntff_profile_hook = lambda: state["h"]
    mod.set_axon_ntff_profile_hook = lambda h: state.__setitem__("h", h)
    sys.modules["antenv.axon_hooks"] = mod
    antenv.axon_hooks = mod


def run_traced(inputs, **kw):
    """For test.py: run with NTFF profiling; returns (output, BassKernelResults)."""
    from concourse.bass_utils import run_bass_kernel_spmd

    _ensure_ntff_hook()

    in_maps, r0 = _prep(**inputs)
    nc = _get_compiled()
    res = run_bass_kernel_spmd(nc, in_maps, core_ids=list(range(NCORES)), trace=True, **kw)
    xf = np.asarray(inputs["x"], dtype=np.float32).reshape(B, N, C)
    return _gather(res.results, xf, r0), res


# revision 47
# speedup vs baseline: 1.0177x; 1.0177x over previous
"""Trainium2 Bass kernel for nn_AttentionBlock (B=4, H=W=64, C=256, D=32).

Sharding: 8 shards = 4 samples x 2 query-halves. Each core gets the full
sample's rows (reordered so its 2048 query rows come first), computes K for
all 4096 keys, and attention for its 2048 queries. No collectives.

v6 algorithm (projection folding + fp8 end-to-end):
  device: dev[q,c] = (1/d[q]) * (G^T @ W2)[q,c];  host: out = x + dev
  W2 = 32 * wv @ wo   (host precompute)
  G[c,q] = sum_k x8[k,c] * E8[k,q]   (fp8 DoubleRow matmuls, contraction 256)
  E8 = fp8e5m2(exp(S/256 - 2)),  S = K Q^T scores at x256 scale (wq,wk
       stored e4m3 with x16 scale each; compensated in the exp scale)
  d[q] = sum_k E8[k,q]       (col-packed ones matmuls + transpose matmul)
exp computed two ways in parallel: ACT true exp -> e5m2, and DVE integer
bit-trick (Schraudolph in e5m2 space: bits = (5.7708/256)*S + 48.76).
S matmuls (contraction D=32) use 4-way PE row-tiling via tile_position.
Q/K projections use DoubleRow. HAM warmup matmuls bridge the ~9us DMA
startup dead zone; phases A/B interleave; epilogue spread across steps.

Self-contained: hardcodes shapes, imports only /opt/trn_rl_repo concourse.
"""

import sys

if "/opt/trn_rl_repo" not in sys.path:
    sys.path.insert(0, "/opt/trn_rl_repo")

import numpy as np
import ml_dtypes

BF16 = ml_dtypes.bfloat16
E4M3 = ml_dtypes.float8_e4m3
E5M2 = ml_dtypes.float8_e5m2

# Problem constants
B, HH, WW, C = 4, 64, 64, 256
D = 32
N = HH * WW           # 4096 keys per sample
NQ = N // 2           # 2048 queries per core
NCORES = 8
KC = N // 128         # 32 key chunks
NG = NQ // 512        # 4 query groups of 512 per core
NSTEP = 8             # 4-chunk steps per query group (32 chunks / 4)
PIPE = 2              # consume s - PIPE

C0 = 4.77             # exp shift: weights = exp(S - C0), cancels in softmax
SS = 256.0            # scores arrive at x256 scale (wq,wk each x16 in fp8)
EXP_A = 5.770780 / SS  # 4*log2(e) / 256
EXP_B = 60.0 + 0.3 - 5.770780 * C0  # e5m2 bias 60, +0.3 truncation recenter
NWARM = 48            # HAM warmup matmuls (bridge ~9us DMA startup)

_compiled_cache = {}


def _build():
    from contextlib import ExitStack
    from concourse import bacc, tile, mybir, masks

    f32 = mybir.dt.float32
    bf = mybir.dt.bfloat16
    fp8e4 = mybir.dt.float8e4
    fp8e5 = mybir.dt.float8e5
    u8 = mybir.dt.uint8

    nc = bacc.Bacc("TRN2", target_bir_lowering=False, debug=False, num_devices=NCORES)

    x8_d = nc.dram_tensor("x8p", [128, KC, 256], fp8e4, kind="ExternalInput")
    xT8_d = nc.dram_tensor("xT8p", [128, 2, N], fp8e4, kind="ExternalInput")
    wq_d = nc.dram_tensor("wq8p", [128, 256], fp8e4, kind="ExternalInput")
    wk_d = nc.dram_tensor("wk8p", [128, 256], fp8e4, kind="ExternalInput")
    bq_d = nc.dram_tensor("bq_col", [128, 1], f32, kind="ExternalInput")
    bk_d = nc.dram_tensor("bk_col", [128, 1], f32, kind="ExternalInput")
    w2_d = nc.dram_tensor("w28p", [128, 512], fp8e4, kind="ExternalInput")
    out_d = nc.dram_tensor("out", [NQ, C], bf, kind="ExternalOutput")

    Exp = mybir.ActivationFunctionType.Exp
    Add = mybir.AluOpType.add
    Mult = mybir.AluOpType.mult
    DR = mybir.MatmulPerfMode.DoubleRow

    with tile.TileContext(nc) as tc:
        with ExitStack() as ctx:
            const = ctx.enter_context(tc.tile_pool(name="const", bufs=1))
            big = ctx.enter_context(tc.tile_pool(name="big", bufs=1))
            expp = ctx.enter_context(tc.tile_pool(name="expp", bufs=12))
            small = ctx.enter_context(tc.tile_pool(name="small", bufs=2))
            ps_s = ctx.enter_context(tc.tile_pool(name="ps_s", bufs=2, space="PSUM"))
            ps_g = ctx.enter_context(tc.tile_pool(name="ps_g", bufs=1, space="PSUM"))
            ps_d = ctx.enter_context(tc.tile_pool(name="ps_d", bufs=1, space="PSUM"))
            ps_e = ctx.enter_context(tc.tile_pool(name="ps_e", bufs=1, space="PSUM"))

            # ---- consts: warmup weights first (plain memset, fast), then identity ----
            warm8 = const.tile([128, 128], fp8e4, tag="warm8")
            nc.gpsimd.memset(warm8[:], 1.0)
            ones8 = const.tile([128, 32], fp8e5, tag="ones8")
            nc.gpsimd.memset(ones8[:], 1.0)
            ones1 = const.tile([128, 1], bf, tag="ones1")
            nc.gpsimd.memset(ones1[:], 1.0)
            negc0 = const.tile([128, 1], f32, tag="negc0")
            nc.gpsimd.memset(negc0[:], -C0)

            # ---- input DMAs: x8 split across both HWDGE rings, fat descriptors ----
            x8sb = big.tile([128, KC, 256], fp8e4, tag="x8sb")
            wqsb = const.tile([128, 2, 128], fp8e4, tag="wqsb")
            wksb = const.tile([128, 2, 128], fp8e4, tag="wksb")
            w2sb = const.tile([128, 2, 256], fp8e4, tag="w2sb")
            bqc = const.tile([128, 1], f32, tag="bqc")
            bkc = const.tile([128, 1], f32, tag="bkc")

            xT8 = big.tile([128, 2, N], fp8e4, tag="xT8")
            nc.sync.dma_start(out=wqsb[:], in_=wq_d[:].rearrange("p (j m) -> p j m", j=2))
            nc.sync.dma_start(out=wksb[:], in_=wk_d[:].rearrange("p (j m) -> p j m", j=2))
            nc.sync.dma_start(out=bqc[:], in_=bq_d[:])
            nc.sync.dma_start(out=bkc[:], in_=bk_d[:])
            nc.sync.dma_start(out=xT8[:, :, 0:2048], in_=xT8_d[:, :, 0:2048])
            nc.sync.dma_start(out=xT8[:, :, 2048:4096], in_=xT8_d[:, :, 2048:4096])
            nc.sync.dma_start(out=x8sb[:, 0:16, :], in_=x8_d[:, 0:16, :])
            nc.sync.dma_start(out=x8sb[:, 16:32, :], in_=x8_d[:, 16:32, :])
            nc.sync.dma_start(out=w2sb[:], in_=w2_d[:].rearrange("p (j m) -> p j m", j=2))

            # ---- HAM warmup: dense dummy matmuls while DMAs land ----
            wmt = ps_s.tile([128, 2, 512], f32, tag="s", name="warm")
            for i in range(NWARM):
                nc.tensor.matmul(wmt[:, 0, 0:128], warm8[:], warm8[:], start=True, stop=True)
            # pre-load ACT exp table
            dumm = const.tile([128, 1], f32, tag="dumm")
            nc.scalar.activation(dumm[:], negc0[:], Exp, bias=negc0[:])

            # ---- phase B: qT/kT replicated x4 along partitions, DoubleRow.
            # Casts split DVE/scalar; filler matmuls keep the PE dense.
            qT = big.tile([128, NQ], bf, tag="qT")
            kT = big.tile([128, N], bf, tag="kT")

            pcount = [0]

            def project(nm, dst, w, bias, s, home):
                # projections alternate between the S pool and the (idle until
                # phase C) denom bank: two independent psum chains pipeline
                if home == "d":
                    pjt = ps_d.tile([128, 512], f32, tag="d", name=f"pj{nm}{s}")
                    pout = pjt[:]
                    filt = None
                else:
                    pjt = ps_s.tile([128, 2, 512], f32, tag="s", name=f"pj{nm}{s}")
                    pout = pjt[:, 0, :]
                    filt = pjt[:, 1, 0:128]
                nc.tensor.matmul(pout, w[:], xT8[:, :, 512 * s : 512 * s + 512],
                                 start=True, stop=True, perf_mode=DR)
                dv = dst[:, 512 * s : 512 * s + 512]
                if pcount[0] % 2 == 0:
                    nc.vector.tensor_scalar(dv, pout, bias, None, Add)
                else:
                    nc.scalar.activation(dv, pout,
                                         mybir.ActivationFunctionType.Identity, bias=bias)
                pcount[0] += 1
                # fillers into the unused psum half: keep the PE dense for HAM
                if filt is not None:
                    for i in range(2):
                        nc.tensor.matmul(filt, warm8[:], warm8[:], start=True, stop=True)

            for s in range(8):
                project("k", kT, wksb, bkc, s, "d" if s % 2 == 1 else "s")
            project("q", qT, wqsb, bqc, 0, "s")

            # ---- phase C: flat pipeline over 32 steps of 4 key chunks ----
            sts = {}
            ets = {}
            gtile = {}
            dtile = {}
            ottile = {}
            epi_state = {}  # group -> (gsb, rec, ott)

            def produce(s):
                g, t = divmod(s, NSTEP)
                if t == 0:
                    gtile[g] = ps_g.tile([128, 2, 512], f32, tag="g", name=f"g{g}")
                    ottile[g] = small.tile([128, 4, 256], bf, tag="ot", name=f"ot{g}")
                if s < 3:
                    project("q", qT, wqsb, bqc, s + 1, "d")
                if s < PIPE + 2:
                    fil = ps_e.tile([128, 256], f32, tag="er", name=f"fil{s}")
                    for i in range(8):
                        nc.tensor.matmul(fil[:, 0:128], warm8[:], warm8[:], start=True, stop=True)
                sA = ps_s.tile([128, 2, 512], f32, tag="s", name=f"sA{s}")
                sB = ps_s.tile([128, 2, 512], f32, tag="s", name=f"sB{s}")
                for i in range(4):
                    m = 4 * t + i
                    nc.tensor.matmul(
                        (sA if i < 2 else sB)[:, i % 2, :],
                        kT[32 * i : 32 * i + 32, 128 * m : 128 * m + 128],
                        qT[32 * i : 32 * i + 32, 512 * g : 512 * g + 512],
                        start=True,
                        stop=True,
                        tile_position=(32 * i, 0),
                    )
                etA = expp.tile([128, 2, 512], fp8e5, tag="e", name=f"eA{s}")
                etB = expp.tile([128, 2, 512], fp8e5, tag="e", name=f"eB{s}")
                nc.scalar.activation(etA[:], sA[:], Exp, bias=negc0[:], scale=1.0 / SS)
                boundary = (s - PIPE) % NSTEP == NSTEP - 1 and s >= PIPE
                if boundary:
                    # epilogue runs on DVE this step: shift exp load to ACT
                    nc.scalar.activation(etB[:, 0, :], sB[:, 0, :], Exp, bias=negc0[:], scale=1.0 / SS)
                    nc.vector.tensor_scalar(
                        etB[:, 1, :].bitcast(u8), sB[:, 1, :], EXP_A, EXP_B, Mult, Add
                    )
                else:
                    nc.vector.tensor_scalar(
                        etB[:].bitcast(u8), sB[:], EXP_A, EXP_B, Mult, Add
                    )
                sts[s] = (sA, sB)
                ets[s] = (etA, etB)

            def consume(s):
                g, t = divmod(s, NSTEP)
                etA, etB = ets.pop(s)
                sts.pop(s)
                gp = gtile[g]
                if t == 0:
                    dtile[g] = ps_d.tile([128, 512], f32, tag="d", name=f"d{g}")
                dp = dtile[g]
                for pa, et in ((0, etA), (1, etB)):
                    pp = 2 * t + pa
                    for h in range(2):
                        nc.tensor.matmul(
                            gp[:, h, :],
                            x8sb[:, 4 * t + 2 * pa : 4 * t + 2 * pa + 2, 128 * h : 128 * h + 128],
                            et[:],
                            start=(pp == 0),
                            stop=(pp == 2 * NSTEP - 1),
                            perf_mode=DR,
                        )
                for j in range(4):
                    et = (etA if j < 2 else etB)
                    nc.tensor.matmul(
                        dp[32 * j : 32 * j + 32, :],
                        ones8[:],
                        et[:, j % 2, :],
                        start=(t == 0),
                        stop=(t == NSTEP - 1),
                        tile_position=(0, 32 * j),
                    )

            def epi_head(g):
                """Group boundary: free G/denom psum via casts, compute rec."""
                gp = gtile.pop(g)
                dp = dtile.pop(g)
                dsb = small.tile([128, 512], bf, tag="dsb")
                nc.vector.tensor_copy(dsb[:], dp[:])
                gsb = small.tile([128, 2, 512], fp8e4, tag="gsb")
                nc.scalar.copy(gsb[:, 0, :], gp[:, 0, :])
                nc.vector.tensor_copy(gsb[:, 1, :], gp[:, 1, :])
                rp = ps_d.tile([128, 512], f32, tag="d", name=f"rp{g}")
                for b in range(4):
                    nc.tensor.matmul(
                        rp[:, b : b + 1], dsb[:, 128 * b : 128 * b + 128],
                        ones1[:], start=True, stop=True,
                    )
                rec = small.tile([128, 4], f32, tag="recs")
                nc.vector.reciprocal(rec[:], rp[:, 0:4])
                epi_state[g] = (gsb, rec, ottile.pop(g), None)

            def epi_block(g, b):
                gsb, rec, ott, eplast = epi_state[g]
                if g == NG - 1:
                    # last group: no later compute hides the chain; rotate
                    # across the er bank and the two freed G banks so the
                    # per-block matmuls pipeline without false deps
                    if eplast is None:
                        eplast = ps_g.tile([128, 2, 512], f32, tag="g", name="glast")
                        epi_state[g] = (gsb, rec, ott, eplast)
                    if b % 3 == 2:
                        ept = ps_e.tile([128, 256], f32, tag="er", name=f"erL{b}")
                        ep = ept[:]
                    else:
                        ep = eplast[:, b % 3, 0:256]
                else:
                    ept = ps_e.tile([128, 256], f32, tag="er", name=f"er{g}_{b}")
                    ep = ept[:]
                nc.tensor.matmul(ep, gsb[:, :, 128 * b : 128 * b + 128], w2sb[:],
                                 start=True, stop=True, perf_mode=DR)
                if b % 2 == 0:
                    nc.vector.tensor_scalar(ott[:, b, :], ep, rec[:, b : b + 1], None, Mult)
                else:
                    nc.scalar.mul(ott[:, b, :], ep, rec[:, b : b + 1])
                out_r = out_d[:].rearrange("(t p) c -> p t c", p=128)
                if g == NG - 1:
                    nc.sync.dma_start(
                        out=out_r[:, 4 * g + b : 4 * g + b + 1, :], in_=ott[:, b : b + 1, :]
                    )
                elif b == 3:
                    nc.sync.dma_start(out=out_r[:, 4 * g : 4 * g + 4, :], in_=ott[:])
                if b == 3:
                    del epi_state[g]

            for s in range(NG * NSTEP + PIPE + 4):
                if s >= PIPE and s - PIPE < NG * NSTEP:
                    sc = s - PIPE
                    consume(sc)
                    if sc % NSTEP == NSTEP - 1:
                        epi_head(sc // NSTEP)
                # spread epilogue blocks: block b of group g at flat-step
                # (boundary of g) + 1 + b; last group bursts immediately
                for g in range(NG - 1):
                    bnd = 8 * g + 7 + PIPE  # flat-step of epi_head(g)
                    b = s - bnd - 1
                    if 0 <= b < 4:
                        epi_block(g, b)
                if s == 8 * (NG - 1) + 7 + PIPE:
                    for b in range(4):
                        epi_block(NG - 1, b)
                if s < NG * NSTEP:
                    produce(s)

    nc.compile()
    return nc


def _get_compiled():
    if "v6" not in _compiled_cache:
        _compiled_cache["v6"] = _build()
    return _compiled_cache["v6"]


def _prep(x, wq, bq, wk, bk, wv, bv, wo, bo):
    xf = np.ascontiguousarray(np.asarray(x, dtype=np.float32)).reshape(B, N, C)
    wq = np.asarray(wq, np.float32)
    bq = np.asarray(bq, np.float32)
    wk = np.asarray(wk, np.float32)
    bk = np.asarray(bk, np.float32)
    wv = np.asarray(wv, np.float32)
    bv = np.asarray(bv, np.float32)
    wo = np.asarray(wo, np.float32)
    bo = np.asarray(bo, np.float32)

    scale = np.float32(1.0 / np.sqrt(np.float32(D)))
    # wq,wk stored e4m3 at x16 scale each (S comes out x256; exp rescales),
    # replicated x4 along d, then packed [128, 2, 128] for DoubleRow.
    wq_rep = np.tile(wq * (scale * 16.0), (1, 4)).astype(E4M3)  # [256, 128]
    wk_rep = np.tile(wk * 16.0, (1, 4)).astype(E4M3)
    wq8p = np.ascontiguousarray(wq_rep.reshape(2, 128, 128).transpose(1, 0, 2).reshape(128, 256))
    wk8p = np.ascontiguousarray(wk_rep.reshape(2, 128, 128).transpose(1, 0, 2).reshape(128, 256))
    bq_col = np.ascontiguousarray(np.tile(bq * (scale * 16.0), 4)[:, None]).astype(np.float32)
    bk_col = np.ascontiguousarray(np.tile(bk * 16.0, 4)[:, None]).astype(np.float32)
    w2f = (32.0 * (wv @ wo)).astype(E4M3)  # x32 folds denom replication
    w2 = np.ascontiguousarray(w2f.reshape(2, 128, 256).transpose(1, 0, 2).reshape(128, 512))
    r0 = (bv @ wo + bo).astype(np.float32)  # residual-folded constant bias row

    in_maps = []
    for core in range(NCORES):
        b, h = divmod(core, 2)
        if h == 0:
            xo = xf[b]
        else:
            xo = np.concatenate([xf[b, NQ:], xf[b, :NQ]], 0)
        # partition-major layouts with 8KB contiguous per partition
        xo8 = xo.astype(E4M3)
        x8p = np.ascontiguousarray(xo8.reshape(KC, 128, 256).transpose(1, 0, 2))
        xT8p = np.ascontiguousarray(xo8.T.reshape(2, 128, N).transpose(1, 0, 2))
        in_maps.append(
            {
                "x8p": x8p,
                "xT8p": xT8p,
                "wq8p": wq8p,
                "wk8p": wk8p,
                "bq_col": bq_col,
                "bk_col": bk_col,
                "w28p": w2,
            }
        )
    return in_maps, r0


def _gather(results, xf, r0):
    out = np.empty((B, N, C), np.float32)
    for core in range(NCORES):
        b, h = divmod(core, 2)
        out[b, NQ * h : NQ * (h + 1)] = results[core]["out"].astype(np.float32)
    out += xf + r0[None, None, :]  # residual + folded output bias (host side)
    return out.reshape(B, HH, WW, C)


def kernel(x, wq, bq, wk, bk, wv, bv, wo, bo):
    from concourse.bass_utils import run_bass_kernel_spmd

    in_maps, r0 = _prep(x, wq, bq, wk, bk, wv, bv, wo, bo)
    nc = _get_compiled()
    res = run_bass_kernel_spmd(nc, in_maps, core_ids=list(range(NCORES)))
    xf = np.asarray(x, dtype=np.float32).reshape(B, N, C)
    return _gather(res.results, xf, r0)


def _ensure_ntff_hook():
    """The agent image's antenv stub lacks axon_hooks; synthesize it so
    run_bass_kernel_spmd(trace=True) can NTFF-profile via libaxon_pjrt."""
    import types

    try:
        from antenv.axon_hooks import get_axon_ntff_profile_hook  # noqa: F401
        return
    except ImportError:
        pass
    import antenv
    from trn_agent_boot.trn_boot import _ntff_profile_via_ctypes

    mod = types.ModuleType("antenv.axon_hooks")
    state = {"h": _ntff_profile_via_ctypes("/opt/axon/libaxon_pjrt.so")}
    mod.get_axon_ntff_profile_hook = lambda: state["h"]
    mod.set_axon_ntff_profile_hook = lambda h: state.__setitem__("h", h)
    sys.modules["antenv.axon_hooks"] = mod
    antenv.axon_hooks = mod


def run_traced(inputs, **kw):
    """For test.py: run with NTFF profiling; returns (output, BassKernelResults)."""
    from concourse.bass_utils import run_bass_kernel_spmd

    _ensure_ntff_hook()

    in_maps, r0 = _prep(**inputs)
    nc = _get_compiled()
    res = run_bass_kernel_spmd(nc, in_maps, core_ids=list(range(NCORES)), trace=True, **kw)
    xf = np.asarray(inputs["x"], dtype=np.float32).reshape(B, N, C)
    return _gather(res.results, xf, r0), res
